# revision 1
# baseline (speedup 1.0000x reference)
import numpy as np

# nn_AxialAttention_32152125177891 — hardcoded problem shapes
MAX_SPAN = 255
N, C, L = 896, 1024, 56
H, KD, VD = 8, 512, 1024
DK, DV = KD // H, VD // H  # 64, 128
EPS = 1e-3


def _dist():
    idx = np.arange(L)
    return (idx[None, :] - idx[:, None] + MAX_SPAN - 1).astype(np.int64)  # [L, L]


def _bn_affine(x, g, b, axes):
    """Return per-channel scale/shift for training-mode BN (biased var).

    Stats computed in float64 for robustness, applied in float32.
    x: array with channel on axis 1; axes: reduction axes (all but 1).
    """
    mean = x.mean(axis=axes, dtype=np.float64)
    var = (x.astype(np.float64) ** 2).mean(axis=axes) - mean ** 2
    var = np.maximum(var, 0.0)
    s = (g.astype(np.float64) / np.sqrt(var + EPS)).astype(np.float32)
    t = (b.astype(np.float64) - mean * (g.astype(np.float64) / np.sqrt(var + EPS))).astype(np.float32)
    return s, t


def kernel(x, w_qkv, bn_qkv_g, bn_qkv_b, q_rpe, k_rpe, v_rpe,
           bn_sim_g, bn_sim_b, bn_out_g, bn_out_b):
    x = np.asarray(x, dtype=np.float32)
    w_qkv = np.asarray(w_qkv, dtype=np.float32)
    bn_qkv_g = np.asarray(bn_qkv_g, dtype=np.float32)
    bn_qkv_b = np.asarray(bn_qkv_b, dtype=np.float32)
    q_rpe = np.asarray(q_rpe, dtype=np.float32)
    k_rpe = np.asarray(k_rpe, dtype=np.float32)
    v_rpe = np.asarray(v_rpe, dtype=np.float32)
    bn_sim_g = np.asarray(bn_sim_g, dtype=np.float32)
    bn_sim_b = np.asarray(bn_sim_b, dtype=np.float32)
    bn_out_g = np.asarray(bn_out_g, dtype=np.float32)
    bn_out_b = np.asarray(bn_out_b, dtype=np.float32)

    n = x.shape[0]
    D = _dist()  # [L, L]

    # ---- qkv projection: one big GEMM [2KD+VD, C] @ [C, n*L] ----
    xm = np.ascontiguousarray(x.transpose(1, 0, 2)).reshape(C, n * L)
    qkv = (w_qkv @ xm).reshape(2 * KD + VD, n, L)  # [O, n, L]

    # batchnorm over (n, L) per output channel
    s0, t0 = _bn_affine(qkv.transpose(1, 0, 2), bn_qkv_g, bn_qkv_b, axes=(0, 2))
    qkv = qkv * s0[:, None, None] + t0[:, None, None]

    q = qkv[:KD].reshape(H, DK, n, L)          # [H, DK, n, L]
    k = qkv[KD:2 * KD].reshape(H, DK, n, L)
    v = qkv[2 * KD:].reshape(H, DV, n, L)

    # ---- similarity logits ----
    # content[b,h,l,m] = sum_d q[h,d,b,l] k[h,d,b,m]
    qb = np.ascontiguousarray(q.transpose(2, 0, 3, 1)).reshape(n * H, L, DK)  # [bh, l, d]
    kb = np.ascontiguousarray(k.transpose(2, 0, 1, 3)).reshape(n * H, DK, L)  # [bh, d, m]
    content = np.matmul(qb, kb).reshape(n, H, L, L)

    qr = q_rpe[D]  # [L, L, DK] (l, m, d)
    kr = k_rpe[D]

    # q_sim[b,h,l,m] = sum_d q[b,h,d,l] * qr[l,m,d]  -> batch over l
    q_l = np.ascontiguousarray(qb.transpose(1, 0, 2))          # [l, bh, d]
    q_sim = np.matmul(q_l, qr.transpose(0, 2, 1))              # [l, bh, m]
    q_sim = q_sim.transpose(1, 0, 2).reshape(n, H, L, L)

    # k_sim[b,h,l,m] = sum_d k[b,h,d,m] * kr[l,m,d]  -> batch over m
    k_m = np.ascontiguousarray(kb.transpose(2, 0, 1))          # [m, bh, d]
    kr_m = np.ascontiguousarray(kr.transpose(1, 2, 0))         # [m, d, l]
    k_sim = np.matmul(k_m, kr_m)                               # [m, bh, l]
    k_sim = k_sim.transpose(1, 2, 0).reshape(n, H, L, L)

    # BN over 3H channels jointly, then sum the three groups
    sims = (content, q_sim, k_sim)
    cnt = float(n * L * L)
    s_list, t_list = [], []
    for gi, arr in enumerate(sims):
        g = bn_sim_g[gi * H:(gi + 1) * H]
        b = bn_sim_b[gi * H:(gi + 1) * H]
        mean = arr.mean(axis=(0, 2, 3), dtype=np.float64)
        var = (arr.astype(np.float64) ** 2).reshape(n, H, -1).mean(axis=(0, 2)) - mean ** 2
        var = np.maximum(var, 0.0)
        sc = g.astype(np.float64) / np.sqrt(var + EPS)
        s_list.append(sc.astype(np.float32))
        t_list.append((b.astype(np.float64) - mean * sc).astype(np.float32))

    sim = (content * s_list[0][None, :, None, None] + t_list[0][None, :, None, None]
           + q_sim * s_list[1][None, :, None, None] + t_list[1][None, :, None, None]
           + k_sim * s_list[2][None, :, None, None] + t_list[2][None, :, None, None])

    # softmax over last axis
    sim -= sim.max(axis=-1, keepdims=True)
    np.exp(sim, out=sim)
    sim /= sim.sum(axis=-1, keepdims=True)
    w = sim  # [n, H, L, L]

    # ---- value retrieval ----
    vb = np.ascontiguousarray(v.transpose(2, 0, 3, 1)).reshape(n * H, L, DV)  # [bh, m, dv]
    wb = w.reshape(n * H, L, L)                                               # [bh, l, m]
    rc = np.matmul(wb, vb)                                                    # [bh, l, dv]
    rc = rc.reshape(n, H, L, DV).transpose(0, 1, 3, 2)                        # [n,H,DV,L]

    vr = v_rpe[D]  # [L, L, DV] (l, m, d)
    w_l = np.ascontiguousarray(wb.transpose(1, 0, 2))  # [l, bh, m]
    rr = np.matmul(w_l, vr)                            # [l, bh, dv]
    rr = rr.transpose(1, 2, 0).reshape(n, H, DV, L)    # [n,H,DV,L]

    # out BN over 2*VD channels, then sum halves
    rc2 = rc.reshape(n, VD, L)
    rr2 = rr.reshape(n, VD, L)
    s1, t1 = _bn_affine(rc2, bn_out_g[:VD], bn_out_b[:VD], axes=(0, 2))
    s2, t2 = _bn_affine(rr2, bn_out_g[VD:], bn_out_b[VD:], axes=(0, 2))
    out = (rc2 * s1[None, :, None] + t1[None, :, None]
           + rr2 * s2[None, :, None] + t2[None, :, None])
    return out.astype(np.float32)



# revision 2
# speedup vs baseline: 1.1545x; 1.1545x over previous
import numpy as np
import jax
import jax.numpy as jnp
from functools import partial

# nn_AxialAttention_32152125177891 — hardcoded problem shapes.
# Strategy: data-parallel over N across the 8 NeuronCores (pmap);
# SyncBatchNorm statistics via lax.psum; RPE gather tables precomputed
# host-side (avoids the broken on-device gather path). All device compute
# in f32; bulk tensors cross the axon tunnel as fp16 (transfer-bound).
MAX_SPAN = 255
N, C, L = 896, 1024, 56
H, KD, VD = 8, 512, 1024
DK, DV = KD // H, VD // H  # 64, 128
EPS = 1e-3
NCORES = 8
B = N // NCORES  # 112 batches per core


def _dist():
    idx = np.arange(L)
    return (idx[None, :] - idx[:, None] + MAX_SPAN - 1).astype(np.int32)  # [L, L]


def _bn_apply(arr, g, b, tot_sum, tot_sumsq, cnt, ax):
    """arr: local shard; tot_sum/tot_sumsq: global per-channel stats (psummed)."""
    mean = tot_sum / cnt
    var = tot_sumsq / cnt - mean * mean
    var = jnp.maximum(var, 0.0)
    scale = jax.lax.rsqrt(var + EPS) * g
    shift = b - mean * scale
    shp = [1] * arr.ndim
    shp[ax] = -1
    return arr * scale.reshape(shp) + shift.reshape(shp)


@partial(jax.pmap, axis_name="i",
         in_axes=(0, None, None, None, None, None, None, None, None, None))
def _run(x, w_qkv, bn_qkv_g, bn_qkv_b, qr, kr, vr,
         bn_sim_gb, bn_out_g, bn_out_b):
    x = x.astype(jnp.float32)
    w_qkv = w_qkv.astype(jnp.float32)
    qr = qr.astype(jnp.float32)
    kr = kr.astype(jnp.float32)
    vr = vr.astype(jnp.float32)

    # 1x1 conv == channel matmul: [O, C] @ [B, C, L] -> [B, O, L]
    qkv = jnp.einsum('oc,bcl->bol', w_qkv, x)

    # global BN over (n, l) per channel
    s = jax.lax.psum(jnp.sum(qkv, axis=(0, 2)), "i")
    ss = jax.lax.psum(jnp.sum(qkv * qkv, axis=(0, 2)), "i")
    qkv = _bn_apply(qkv, bn_qkv_g, bn_qkv_b, s, ss, float(N * L), ax=1)

    q = qkv[:, :KD].reshape(B, H, DK, L)
    k = qkv[:, KD:2 * KD].reshape(B, H, DK, L)
    v = qkv[:, 2 * KD:].reshape(B, H, DV, L)

    content = jnp.einsum('bhdl,bhdm->bhlm', q, k)
    q_sim = jnp.einsum('bhdl,lmd->bhlm', q, qr)
    k_sim = jnp.einsum('bhdm,lmd->bhlm', k, kr)

    def bn_sim(arr, gi):
        g = bn_sim_gb[0, gi * H:(gi + 1) * H]
        b = bn_sim_gb[1, gi * H:(gi + 1) * H]
        s = jax.lax.psum(jnp.sum(arr, axis=(0, 2, 3)), "i")
        ss = jax.lax.psum(jnp.sum(arr * arr, axis=(0, 2, 3)), "i")
        return _bn_apply(arr, g, b, s, ss, float(N * L * L), ax=1)

    sim = bn_sim(content, 0) + bn_sim(q_sim, 1) + bn_sim(k_sim, 2)
    w = jax.nn.softmax(sim, axis=-1)  # [B, H, L, L]

    rc = jnp.einsum('bhlm,bhdm->bhdl', w, v).reshape(B, VD, L)
    rr = jnp.einsum('bhlm,lmd->bhdl', w, vr).reshape(B, VD, L)

    def bn_out(arr, g, b):
        s = jax.lax.psum(jnp.sum(arr, axis=(0, 2)), "i")
        ss = jax.lax.psum(jnp.sum(arr * arr, axis=(0, 2)), "i")
        return _bn_apply(arr, g, b, s, ss, float(N * L), ax=1)

    out = bn_out(rc, bn_out_g[:VD], bn_out_b[:VD]) \
        + bn_out(rr, bn_out_g[VD:], bn_out_b[VD:])
    return out.astype(jnp.float16)  # fp16 on the wire


def kernel(x, w_qkv, bn_qkv_g, bn_qkv_b, q_rpe, k_rpe, v_rpe,
           bn_sim_g, bn_sim_b, bn_out_g, bn_out_b):
    x = np.asarray(x, dtype=np.float32)
    D = _dist()
    qr = np.asarray(q_rpe, dtype=np.float32)[D].astype(np.float16)  # [L, L, DK]
    kr = np.asarray(k_rpe, dtype=np.float32)[D].astype(np.float16)
    vr = np.asarray(v_rpe, dtype=np.float32)[D].astype(np.float16)  # [L, L, DV]

    x_sh = x.reshape(NCORES, B, C, L).astype(np.float16)
    bn_sim_gb = np.stack([np.asarray(bn_sim_g, np.float32),
                          np.asarray(bn_sim_b, np.float32)])
    out = _run(x_sh,
               np.asarray(w_qkv, np.float32).astype(np.float16),
               np.asarray(bn_qkv_g, np.float32),
               np.asarray(bn_qkv_b, np.float32),
               qr, kr, vr, bn_sim_gb,
               np.asarray(bn_out_g, np.float32),
               np.asarray(bn_out_b, np.float32))
    return np.asarray(out).reshape(N, VD, L).astype(np.float32)


# revision 4
# speedup vs baseline: 1.2588x; 1.0903x over previous
import numpy as np
import jax
import jax.numpy as jnp
from functools import partial

# nn_AxialAttention_32152125177891 — hardcoded problem shapes.
# Strategy: data-parallel over N across the 8 NeuronCores (pmap);
# SyncBatchNorm statistics via lax.psum; RPE gather tables precomputed
# host-side (avoids the broken on-device gather path). All device compute
# in f32; bulk tensors cross the axon tunnel as fp16 (transfer-bound).
MAX_SPAN = 255
N, C, L = 896, 1024, 56
H, KD, VD = 8, 512, 1024
DK, DV = KD // H, VD // H  # 64, 128
EPS = 1e-3
NCORES = 8
B = N // NCORES  # 112 batches per core


def _dist():
    idx = np.arange(L)
    return (idx[None, :] - idx[:, None] + MAX_SPAN - 1).astype(np.int32)  # [L, L]


def _bn_apply(arr, g, b, tot_sum, tot_sumsq, cnt, ax):
    """arr: local shard; tot_sum/tot_sumsq: global per-channel stats (psummed)."""
    mean = tot_sum / cnt
    var = tot_sumsq / cnt - mean * mean
    var = jnp.maximum(var, 0.0)
    scale = jax.lax.rsqrt(var + EPS) * g
    shift = b - mean * scale
    shp = [1] * arr.ndim
    shp[ax] = -1
    return arr * scale.reshape(shp) + shift.reshape(shp)


@partial(jax.pmap, axis_name="i",
         in_axes=(0, 0, None, None, 0, 0, 0, None, None, None))
def _run(x, w_qkv, bn_qkv_g, bn_qkv_b, qr, kr, vr,
         bn_sim_gb, bn_out_g, bn_out_b):
    x = x.astype(jnp.float32)
    # large constants arrive sharded over the slow host link; reassemble
    # over the fast on-chip interconnect
    w_qkv = jax.lax.all_gather(w_qkv, "i", axis=0, tiled=True).astype(jnp.float32)
    qr = jax.lax.all_gather(qr, "i", axis=0, tiled=True).astype(jnp.float32)
    kr = jax.lax.all_gather(kr, "i", axis=0, tiled=True).astype(jnp.float32)
    vr = jax.lax.all_gather(vr, "i", axis=0, tiled=True).astype(jnp.float32)

    # 1x1 conv == channel matmul: [O, C] @ [B, C, L] -> [B, O, L]
    qkv = jnp.einsum('oc,bcl->bol', w_qkv, x)

    # global BN over (n, l) per channel
    s = jax.lax.psum(jnp.sum(qkv, axis=(0, 2)), "i")
    ss = jax.lax.psum(jnp.sum(qkv * qkv, axis=(0, 2)), "i")
    qkv = _bn_apply(qkv, bn_qkv_g, bn_qkv_b, s, ss, float(N * L), ax=1)

    q = qkv[:, :KD].reshape(B, H, DK, L)
    k = qkv[:, KD:2 * KD].reshape(B, H, DK, L)
    v = qkv[:, 2 * KD:].reshape(B, H, DV, L)

    content = jnp.einsum('bhdl,bhdm->bhlm', q, k)
    q_sim = jnp.einsum('bhdl,lmd->bhlm', q, qr)
    k_sim = jnp.einsum('bhdm,lmd->bhlm', k, kr)

    def bn_sim(arr, gi):
        g = bn_sim_gb[0, gi * H:(gi + 1) * H]
        b = bn_sim_gb[1, gi * H:(gi + 1) * H]
        s = jax.lax.psum(jnp.sum(arr, axis=(0, 2, 3)), "i")
        ss = jax.lax.psum(jnp.sum(arr * arr, axis=(0, 2, 3)), "i")
        return _bn_apply(arr, g, b, s, ss, float(N * L * L), ax=1)

    sim = bn_sim(content, 0) + bn_sim(q_sim, 1) + bn_sim(k_sim, 2)
    w = jax.nn.softmax(sim, axis=-1)  # [B, H, L, L]

    rc = jnp.einsum('bhlm,bhdm->bhdl', w, v).reshape(B, VD, L)
    rr = jnp.einsum('bhlm,lmd->bhdl', w, vr).reshape(B, VD, L)

    def bn_out(arr, g, b):
        s = jax.lax.psum(jnp.sum(arr, axis=(0, 2)), "i")
        ss = jax.lax.psum(jnp.sum(arr * arr, axis=(0, 2)), "i")
        return _bn_apply(arr, g, b, s, ss, float(N * L), ax=1)

    out = bn_out(rc, bn_out_g[:VD], bn_out_b[:VD]) \
        + bn_out(rr, bn_out_g[VD:], bn_out_b[VD:])
    return out.astype(jnp.float16)  # fp16 on the wire


def kernel(x, w_qkv, bn_qkv_g, bn_qkv_b, q_rpe, k_rpe, v_rpe,
           bn_sim_g, bn_sim_b, bn_out_g, bn_out_b):
    x = np.asarray(x, dtype=np.float32)
    D = _dist()
    # host-side RPE gather, sharded over l across cores (all_gathered on device)
    qr = np.asarray(q_rpe, dtype=np.float32)[D].astype(np.float16)  # [L, L, DK]
    kr = np.asarray(k_rpe, dtype=np.float32)[D].astype(np.float16)
    vr = np.asarray(v_rpe, dtype=np.float32)[D].astype(np.float16)  # [L, L, DV]
    qr = qr.reshape(NCORES, L // NCORES, L, DK)
    kr = kr.reshape(NCORES, L // NCORES, L, DK)
    vr = vr.reshape(NCORES, L // NCORES, L, DV)

    x_sh = x.reshape(NCORES, B, C, L).astype(np.float16)
    bn_sim_gb = np.stack([np.asarray(bn_sim_g, np.float32),
                          np.asarray(bn_sim_b, np.float32)])
    out = _run(x_sh,
               np.asarray(w_qkv, np.float32).astype(np.float16)
                 .reshape(NCORES, (2 * KD + VD) // NCORES, C),
               np.asarray(bn_qkv_g, np.float32),
               np.asarray(bn_qkv_b, np.float32),
               qr, kr, vr, bn_sim_gb,
               np.asarray(bn_out_g, np.float32),
               np.asarray(bn_out_b, np.float32))
    return np.asarray(out).reshape(N, VD, L).astype(np.float32)


# revision 7
# speedup vs baseline: 1.3043x; 1.0361x over previous
import numpy as np
import jax
import jax.numpy as jnp
from functools import partial
from concurrent.futures import ThreadPoolExecutor

# nn_AxialAttention_32152125177891 — hardcoded problem shapes.
# Strategy: data-parallel over N across the 8 NeuronCores (pmap);
# SyncBatchNorm statistics via lax.psum; RPE gather tables precomputed
# host-side (avoids the broken on-device gather path). All device compute
# in f32; bulk tensors cross the axon tunnel as fp16 (transfer-bound).
MAX_SPAN = 255
N, C, L = 896, 1024, 56
H, KD, VD = 8, 512, 1024
DK, DV = KD // H, VD // H  # 64, 128
EPS = 1e-3
NCORES = 8
B = N // NCORES  # 112 batches per core


def _dist():
    idx = np.arange(L)
    return (idx[None, :] - idx[:, None] + MAX_SPAN - 1).astype(np.int32)  # [L, L]


def _bn_apply(arr, g, b, tot_sum, tot_sumsq, cnt, ax):
    """arr: local shard; tot_sum/tot_sumsq: global per-channel stats (psummed)."""
    mean = tot_sum / cnt
    var = tot_sumsq / cnt - mean * mean
    var = jnp.maximum(var, 0.0)
    scale = jax.lax.rsqrt(var + EPS) * g
    shift = b - mean * scale
    shp = [1] * arr.ndim
    shp[ax] = -1
    return arr * scale.reshape(shp) + shift.reshape(shp)


@partial(jax.pmap, axis_name="i",
         in_axes=(0, 0, None, None, 0, 0, 0, None, None, None))
def _run(x, w_qkv, bn_qkv_g, bn_qkv_b, qr, kr, vr,
         bn_sim_gb, bn_out_g, bn_out_b):
    x = x.astype(jnp.float32)
    # large constants arrive sharded over the slow host link; reassemble
    # over the fast on-chip interconnect
    w_qkv = jax.lax.all_gather(w_qkv, "i", axis=0, tiled=True).astype(jnp.float32)
    qr = jax.lax.all_gather(qr, "i", axis=0, tiled=True).astype(jnp.float32)
    kr = jax.lax.all_gather(kr, "i", axis=0, tiled=True).astype(jnp.float32)
    vr = jax.lax.all_gather(vr, "i", axis=0, tiled=True).astype(jnp.float32)

    # 1x1 conv == channel matmul: [O, C] @ [B, C, L] -> [B, O, L]
    qkv = jnp.einsum('oc,bcl->bol', w_qkv, x)

    # global BN over (n, l) per channel
    s = jax.lax.psum(jnp.sum(qkv, axis=(0, 2)), "i")
    ss = jax.lax.psum(jnp.sum(qkv * qkv, axis=(0, 2)), "i")
    qkv = _bn_apply(qkv, bn_qkv_g, bn_qkv_b, s, ss, float(N * L), ax=1)

    q = qkv[:, :KD].reshape(B, H, DK, L)
    k = qkv[:, KD:2 * KD].reshape(B, H, DK, L)
    v = qkv[:, 2 * KD:].reshape(B, H, DV, L)

    content = jnp.einsum('bhdl,bhdm->bhlm', q, k)
    q_sim = jnp.einsum('bhdl,lmd->bhlm', q, qr)
    k_sim = jnp.einsum('bhdm,lmd->bhlm', k, kr)

    def bn_sim(arr, gi):
        g = bn_sim_gb[0, gi * H:(gi + 1) * H]
        b = bn_sim_gb[1, gi * H:(gi + 1) * H]
        s = jax.lax.psum(jnp.sum(arr, axis=(0, 2, 3)), "i")
        ss = jax.lax.psum(jnp.sum(arr * arr, axis=(0, 2, 3)), "i")
        return _bn_apply(arr, g, b, s, ss, float(N * L * L), ax=1)

    sim = bn_sim(content, 0) + bn_sim(q_sim, 1) + bn_sim(k_sim, 2)
    w = jax.nn.softmax(sim, axis=-1)  # [B, H, L, L]

    rc = jnp.einsum('bhlm,bhdm->bhdl', w, v).reshape(B, VD, L)
    rr = jnp.einsum('bhlm,lmd->bhdl', w, vr).reshape(B, VD, L)

    def bn_out(arr, g, b):
        s = jax.lax.psum(jnp.sum(arr, axis=(0, 2)), "i")
        ss = jax.lax.psum(jnp.sum(arr * arr, axis=(0, 2)), "i")
        return _bn_apply(arr, g, b, s, ss, float(N * L), ax=1)

    out = bn_out(rc, bn_out_g[:VD], bn_out_b[:VD]) \
        + bn_out(rr, bn_out_g[VD:], bn_out_b[VD:])
    return out.astype(jnp.float16)  # fp16 on the wire


def kernel(x, w_qkv, bn_qkv_g, bn_qkv_b, q_rpe, k_rpe, v_rpe,
           bn_sim_g, bn_sim_b, bn_out_g, bn_out_b):
    x = np.asarray(x, dtype=np.float32)
    D = _dist()
    # host-side RPE gather, sharded over l across cores (all_gathered on device)
    qr = np.asarray(q_rpe, dtype=np.float32)[D].astype(np.float16)  # [L, L, DK]
    kr = np.asarray(k_rpe, dtype=np.float32)[D].astype(np.float16)
    vr = np.asarray(v_rpe, dtype=np.float32)[D].astype(np.float16)  # [L, L, DV]
    qr = qr.reshape(NCORES, L // NCORES, L, DK)
    kr = kr.reshape(NCORES, L // NCORES, L, DK)
    vr = vr.reshape(NCORES, L // NCORES, L, DV)

    x_sh = x.reshape(NCORES, B, C, L).astype(np.float16)
    # mildly parallel h2d of the x shards beats pmap's internal transfer
    try:
        devs = jax.devices()[:NCORES]
        ex = ThreadPoolExecutor(3)
        shards = list(ex.map(lambda i: jax.device_put(x_sh[i], devs[i]),
                             range(NCORES)))
        x_dev = jax.device_put_sharded(shards, devs)
    except Exception:
        x_dev = x_sh
    bn_sim_gb = np.stack([np.asarray(bn_sim_g, np.float32),
                          np.asarray(bn_sim_b, np.float32)])
    out = _run(x_dev,
               np.asarray(w_qkv, np.float32).astype(np.float16)
                 .reshape(NCORES, (2 * KD + VD) // NCORES, C),
               np.asarray(bn_qkv_g, np.float32),
               np.asarray(bn_qkv_b, np.float32),
               qr, kr, vr, bn_sim_gb,
               np.asarray(bn_out_g, np.float32),
               np.asarray(bn_out_b, np.float32))
    # mildly parallel d2h fetch of the output shards
    try:
        ex = ThreadPoolExecutor(3)
        sh = sorted(out.addressable_shards, key=lambda s: s.index[0].start or 0)
        parts = list(ex.map(lambda s: np.asarray(s.data), sh))
        res = np.concatenate(parts, axis=0)
    except Exception:
        res = np.asarray(out)
    return res.reshape(N, VD, L).astype(np.float32)


# revision 10
# speedup vs baseline: 1.3329x; 1.0219x over previous
import numpy as np
import jax
import jax.numpy as jnp
from functools import partial

# nn_AxialAttention_32152125177891 — hardcoded problem shapes.
# Strategy: data-parallel over N across the 8 NeuronCores (pmap);
# SyncBatchNorm statistics via lax.psum; RPE gather tables precomputed
# host-side (avoids the broken on-device gather path). All device compute
# in f32; bulk tensors cross the axon tunnel as fp16 (transfer-bound).
MAX_SPAN = 255
N, C, L = 896, 1024, 56
H, KD, VD = 8, 512, 1024
DK, DV = KD // H, VD // H  # 64, 128
EPS = 1e-3
NCORES = 8
B = N // NCORES  # 112 batches per core


def _dist():
    idx = np.arange(L)
    return (idx[None, :] - idx[:, None] + MAX_SPAN - 1).astype(np.int32)  # [L, L]


def _bn_apply(arr, g, b, tot_sum, tot_sumsq, cnt, ax):
    """arr: local shard; tot_sum/tot_sumsq: global per-channel stats (psummed)."""
    mean = tot_sum / cnt
    var = tot_sumsq / cnt - mean * mean
    var = jnp.maximum(var, 0.0)
    scale = jax.lax.rsqrt(var + EPS) * g
    shift = b - mean * scale
    shp = [1] * arr.ndim
    shp[ax] = -1
    return arr * scale.reshape(shp) + shift.reshape(shp)


@partial(jax.pmap, axis_name="i",
         in_axes=(0, 0, None, None, 0, 0, 0, None, None, None))
def _run(x, w_qkv, bn_qkv_g, bn_qkv_b, qr, kr, vr,
         bn_sim_gb, bn_out_g, bn_out_b):
    x = x.astype(jnp.float32)
    # large constants arrive sharded over the slow host link; reassemble
    # over the fast on-chip interconnect
    w_qkv = jax.lax.all_gather(w_qkv, "i", axis=0, tiled=True).astype(jnp.float32)
    qr = jax.lax.all_gather(qr, "i", axis=0, tiled=True).astype(jnp.float32)
    kr = jax.lax.all_gather(kr, "i", axis=0, tiled=True).astype(jnp.float32)
    vr = jax.lax.all_gather(vr, "i", axis=0, tiled=True).astype(jnp.float32)

    # 1x1 conv == channel matmul: [O, C] @ [B, C, L] -> [B, O, L]
    qkv = jnp.einsum('oc,bcl->bol', w_qkv, x)

    # global BN over (n, l) per channel
    s = jax.lax.psum(jnp.sum(qkv, axis=(0, 2)), "i")
    ss = jax.lax.psum(jnp.sum(qkv * qkv, axis=(0, 2)), "i")
    qkv = _bn_apply(qkv, bn_qkv_g, bn_qkv_b, s, ss, float(N * L), ax=1)

    q = qkv[:, :KD].reshape(B, H, DK, L)
    k = qkv[:, KD:2 * KD].reshape(B, H, DK, L)
    v = qkv[:, 2 * KD:].reshape(B, H, DV, L)

    content = jnp.einsum('bhdl,bhdm->bhlm', q, k)
    q_sim = jnp.einsum('bhdl,lmd->bhlm', q, qr)
    k_sim = jnp.einsum('bhdm,lmd->bhlm', k, kr)

    def bn_sim(arr, gi):
        g = bn_sim_gb[0, gi * H:(gi + 1) * H]
        b = bn_sim_gb[1, gi * H:(gi + 1) * H]
        s = jax.lax.psum(jnp.sum(arr, axis=(0, 2, 3)), "i")
        ss = jax.lax.psum(jnp.sum(arr * arr, axis=(0, 2, 3)), "i")
        return _bn_apply(arr, g, b, s, ss, float(N * L * L), ax=1)

    sim = bn_sim(content, 0) + bn_sim(q_sim, 1) + bn_sim(k_sim, 2)
    w = jax.nn.softmax(sim, axis=-1)  # [B, H, L, L]

    rc = jnp.einsum('bhlm,bhdm->bhdl', w, v).reshape(B, VD, L)
    rr = jnp.einsum('bhlm,lmd->bhdl', w, vr).reshape(B, VD, L)

    def bn_out(arr, g, b):
        s = jax.lax.psum(jnp.sum(arr, axis=(0, 2)), "i")
        ss = jax.lax.psum(jnp.sum(arr * arr, axis=(0, 2)), "i")
        return _bn_apply(arr, g, b, s, ss, float(N * L), ax=1)

    out = bn_out(rc, bn_out_g[:VD], bn_out_b[:VD]) \
        + bn_out(rr, bn_out_g[VD:], bn_out_b[VD:])
    return out.astype(jnp.float16)  # fp16 on the wire


def kernel(x, w_qkv, bn_qkv_g, bn_qkv_b, q_rpe, k_rpe, v_rpe,
           bn_sim_g, bn_sim_b, bn_out_g, bn_out_b):
    x = np.asarray(x, dtype=np.float32)
    D = _dist()
    # host-side RPE gather, sharded over l across cores (all_gathered on device)
    qr = np.asarray(q_rpe, dtype=np.float32)[D].astype(np.float16)  # [L, L, DK]
    kr = np.asarray(k_rpe, dtype=np.float32)[D].astype(np.float16)
    vr = np.asarray(v_rpe, dtype=np.float32)[D].astype(np.float16)  # [L, L, DV]
    qr = qr.reshape(NCORES, L // NCORES, L, DK)
    kr = kr.reshape(NCORES, L // NCORES, L, DK)
    vr = vr.reshape(NCORES, L // NCORES, L, DV)

    x_sh = x.reshape(NCORES, B, C, L).astype(np.float16)
    bn_sim_gb = np.stack([np.asarray(bn_sim_g, np.float32),
                          np.asarray(bn_sim_b, np.float32)])
    out = _run(x_sh,
               np.asarray(w_qkv, np.float32).astype(np.float16)
                 .reshape(NCORES, (2 * KD + VD) // NCORES, C),
               np.asarray(bn_qkv_g, np.float32),
               np.asarray(bn_qkv_b, np.float32),
               qr, kr, vr, bn_sim_gb,
               np.asarray(bn_out_g, np.float32),
               np.asarray(bn_out_b, np.float32))
    return np.asarray(out).reshape(N, VD, L).astype(np.float32)
